# revision 4
# baseline (speedup 1.0000x reference)
"""Trilinear interpolation (BayesianAtlas velocity field advection) on 8 TRN2 cores.

Strategy:
 - Shard batch dim B=16 across 8 cores (2 batch items per core).
 - Host-side layout prep: the velocity grid is repacked into 8 parity-shifted
   "stencil block" layouts. Block (pu,pv,pw, ub,vb,wb) holds the 2x2x2 cell
   stencil with origin (2*ub+pu, 2*vb+pv, 2*wb+pw), channels-last
   ([du,dv,dw,c], 24 floats = 96B contiguous). Any point's 8 corners live in
   exactly one block: origin parity p=i&1, block coord = i>>1.
 - Device: per point, compute grid coords / fractional weights / block id on
   the vector engine, gather its 96B block from DRAM with an indirect DMA
   (one descriptor per point), and do the weighted 8-corner reduction.
"""

import numpy as np

import concourse.bass as bass
import concourse.mybir as mybir
import concourse.tile as tile
from concourse import bacc
from concourse.bass_utils import run_bass_kernel_spmd

G = 128
NB_CORES = 8
B_PER_CORE = 2
N = 200_000
GB = G // 2  # 64 blocks per axis
NBLK = GB * GB * GB  # 262144 blocks per parity layout
ROW = 24  # floats per block (2*2*2*3)

L_BATCH = 1568  # slots per partition per batch item (128*1568 = 200704 >= 200000)
L = 2 * L_BATCH  # 3136 slots per partition per core
NT = 16  # tiles (8 per batch item)
T = L // NT  # 196 points per partition per tile

A = mybir.AluOpType
F32 = mybir.dt.float32
I32 = mybir.dt.int32


def _axis_coords(nc, pool, p_ap, T, tag):
    """From raw coord tile [128,T] -> (frac, i1 int32)."""
    u = pool.tile([128, T], F32, tag=f"{tag}_u")
    # u = (p + 2.5) / 5 * 127 = p*25.4 + 63.5  (fused mult+add)
    nc.vector.tensor_scalar(u[:], p_ap, 25.4, 63.5, op0=A.mult, op1=A.add)
    ti = pool.tile([128, T], I32, tag=f"{tag}_ti")
    tf = pool.tile([128, T], F32, tag=f"{tag}_tf")
    corr = pool.tile([128, T], F32, tag=f"{tag}_corr")
    f1 = pool.tile([128, T], F32, tag=f"{tag}_f1")
    fr = pool.tile([128, T], F32, tag=f"{tag}_fr")
    i1 = pool.tile([128, T], I32, tag=f"{tag}_i1")
    nc.vector.tensor_copy(ti[:], u[:])          # cast (round or trunc)
    nc.vector.tensor_copy(tf[:], ti[:])         # back to float
    nc.vector.tensor_tensor(corr[:], tf[:], u[:], op=A.is_gt)  # 1.0 if too big
    nc.vector.tensor_tensor(f1[:], tf[:], corr[:], op=A.subtract)
    nc.vector.tensor_tensor(fr[:], u[:], f1[:], op=A.subtract)
    nc.vector.tensor_copy(i1[:], f1[:])         # exact int
    return fr, i1


def _build_nc(n_tiles: int):
    nc = bacc.Bacc("TRN2", target_bir_lowering=False, debug=False, enable_asserts=False)

    blocks = nc.dram_tensor(
        "blocks", [B_PER_CORE * 8 * NBLK, ROW], F32, kind="ExternalInput"
    )
    pts = nc.dram_tensor("pts", [3, 128, L], F32, kind="ExternalInput")
    out = nc.dram_tensor("out", [128, L, 3], F32, kind="ExternalOutput")

    tiles_per_b = max(1, n_tiles // B_PER_CORE)

    with tile.TileContext(nc) as tc:
        with (
            tc.tile_pool(name="io", bufs=2) as io_pool,
            tc.tile_pool(name="wk", bufs=2) as wk,
        ):
            for t in range(n_tiles):
                b_item = min(t // tiles_per_b, B_PER_CORE - 1)
                bias = b_item * 8 * NBLK
                sl = slice(t * T, (t + 1) * T)

                pu_ = io_pool.tile([128, T], F32, tag="pu")
                pv_ = io_pool.tile([128, T], F32, tag="pv")
                pw_ = io_pool.tile([128, T], F32, tag="pw")
                nc.sync.dma_start(pu_[:], pts.ap()[0][:, sl])
                nc.sync.dma_start(pv_[:], pts.ap()[1][:, sl])
                nc.sync.dma_start(pw_[:], pts.ap()[2][:, sl])

                du, iu = _axis_coords(nc, wk, pu_[:], T, "x")
                dv, iv = _axis_coords(nc, wk, pv_[:], T, "y")
                dw, iw = _axis_coords(nc, wk, pw_[:], T, "z")

                # block id (int32):
                # bid = ((pu<<2|pv<<1|pw)<<18) | (hu<<12) | (hv<<6) | hw + bias
                #  with p=i&1, h=i>>1
                def par_h(i1, tag):
                    p = wk.tile([128, T], I32, tag=f"{tag}_p")
                    h = wk.tile([128, T], I32, tag=f"{tag}_h")
                    nc.vector.tensor_scalar(p[:], i1[:], 1, None, op0=A.bitwise_and)
                    nc.vector.tensor_scalar(h[:], i1[:], 1, None,
                                            op0=A.logical_shift_right)
                    return p, h

                cu, hu = par_h(iu, "cu")
                cv, hv = par_h(iv, "cv")
                cw, hw = par_h(iw, "cw")

                bid = wk.tile([128, T], I32, tag="bid")
                t0 = wk.tile([128, T], I32, tag="t0")
                # bid = cu*(1<<20) + cv*(1<<19) + cw*(1<<18)
                #     + hu*(1<<12) + hv*(1<<6) + hw + bias
                nc.vector.tensor_scalar(bid[:], cu[:], 1 << 20, bias,
                                        op0=A.mult, op1=A.add)
                nc.vector.scalar_tensor_tensor(t0[:], cv[:], 1 << 19, bid[:],
                                               op0=A.mult, op1=A.add)
                nc.vector.scalar_tensor_tensor(bid[:], cw[:], 1 << 18, t0[:],
                                               op0=A.mult, op1=A.add)
                nc.vector.scalar_tensor_tensor(t0[:], hu[:], 1 << 12, bid[:],
                                               op0=A.mult, op1=A.add)
                nc.vector.scalar_tensor_tensor(bid[:], hv[:], 1 << 6, t0[:],
                                               op0=A.mult, op1=A.add)
                nc.vector.tensor_tensor(t0[:], bid[:], hw[:], op=A.add)

                # --- gather: one 96B block per point
                gt = io_pool.tile([128, T, ROW], F32, tag="gt")
                for j in range(T):
                    nc.gpsimd.indirect_dma_start(
                        out=gt[:, j : j + 1, :].rearrange("p a b -> p (a b)"),
                        out_offset=None,
                        in_=blocks.ap()[:],
                        in_offset=bass.IndirectOffsetOnAxis(
                            ap=t0[:, j : j + 1], axis=0
                        ),
                    )

                # --- corner weights, block order k = du*4 + dv*2 + dw
                gu = wk.tile([128, T], F32, tag="gu")
                gv = wk.tile([128, T], F32, tag="gv")
                gw = wk.tile([128, T], F32, tag="gw")
                nc.vector.tensor_scalar(gu[:], du[:], -1.0,
                                        1.0, op0=A.mult, op1=A.add)
                nc.vector.tensor_scalar(gv[:], dv[:], -1.0,
                                        1.0, op0=A.mult, op1=A.add)
                nc.vector.tensor_scalar(gw[:], dw[:], -1.0,
                                        1.0, op0=A.mult, op1=A.add)
                w_gg = wk.tile([128, T], F32, tag="wgg")
                w_gf = wk.tile([128, T], F32, tag="wgf")
                w_fg = wk.tile([128, T], F32, tag="wfg")
                w_ff = wk.tile([128, T], F32, tag="wff")
                nc.vector.tensor_tensor(w_gg[:], gu[:], gv[:], op=A.mult)
                nc.vector.tensor_tensor(w_gf[:], gu[:], dv[:], op=A.mult)
                nc.vector.tensor_tensor(w_fg[:], du[:], gv[:], op=A.mult)
                nc.vector.tensor_tensor(w_ff[:], du[:], dv[:], op=A.mult)
                w8 = []
                for k, (wuv, wz) in enumerate(
                    [(w_gg, gw), (w_gg, dw), (w_gf, gw), (w_gf, dw),
                     (w_fg, gw), (w_fg, dw), (w_ff, gw), (w_ff, dw)]
                ):
                    wt = wk.tile([128, T], F32, tag=f"w8_{k}")
                    nc.vector.tensor_tensor(wt[:], wuv[:], wz[:], op=A.mult)
                    w8.append(wt)

                # --- reduce: out[p,t,c] = sum_k w8[k][p,t] * gt[p,t,k*3+c]
                ot = io_pool.tile([128, T, 3], F32, tag="ot")
                tmp = wk.tile([128, T, 3], F32, tag="tmp")

                def wview(wt):
                    ap = wt[:]
                    return bass.AP(ap.tensor, ap.offset,
                                   [list(ap.ap[0]), list(ap.ap[1]), [0, 3]])

                nc.vector.tensor_tensor(ot[:], gt[:, :, 0:3], wview(w8[0]),
                                        op=A.mult)
                for k in range(1, 8):
                    nc.vector.tensor_tensor(
                        tmp[:], gt[:, :, 3 * k : 3 * k + 3], wview(w8[k]), op=A.mult
                    )
                    nc.vector.tensor_tensor(ot[:], ot[:], tmp[:], op=A.add)

                nc.sync.dma_start(out.ap()[:, sl, :], ot[:])

    nc.compile()
    return nc


def _host_blocks(vel):
    """vel [B,3,G,G,G] -> list per core of [B_PER_CORE*8*NBLK, ROW] float32."""
    vp = np.pad(vel, ((0, 0), (0, 0), (0, 3), (0, 3), (0, 3)), mode="edge")
    outs = []
    for core in range(NB_CORES):
        v = vp[core * B_PER_CORE : (core + 1) * B_PER_CORE]
        core_blocks = np.empty((B_PER_CORE, 8, GB, GB, GB, 2, 2, 2, 3), np.float32)
        for pu in range(2):
            for pv in range(2):
                for pw in range(2):
                    sel = pu * 4 + pv * 2 + pw
                    sub = v[:, :, pu : pu + 128, pv : pv + 128, pw : pw + 128]
                    sub = sub.reshape(B_PER_CORE, 3, GB, 2, GB, 2, GB, 2)
                    core_blocks[:, sel] = sub.transpose(0, 2, 4, 6, 3, 5, 7, 1)
        outs.append(core_blocks.reshape(B_PER_CORE * 8 * NBLK, ROW))
    return outs


def _host_points(points):
    """points [B, N, 3] -> list per core of [3, 128, L] float32."""
    outs = []
    for core in range(NB_CORES):
        arrs = []
        for bi in range(B_PER_CORE):
            p = points[core * B_PER_CORE + bi]
            pad = 128 * L_BATCH - p.shape[0]
            p = np.concatenate([p, np.zeros((pad, 3), p.dtype)], axis=0)
            arrs.append(p.reshape(128, L_BATCH, 3))
        both = np.concatenate(arrs, axis=1)
        outs.append(np.ascontiguousarray(both.transpose(2, 0, 1)))
    return outs


def kernel(velocity, points, bounding_box, grid_size):
    velocity = np.asarray(velocity, dtype=np.float32)
    points = np.asarray(points, dtype=np.float32)
    bb = np.asarray(bounding_box, dtype=np.float32)
    assert int(grid_size) == G

    # Device math assumes the [-2.5, 2.5] box; remap general boxes on host
    # (identity for the canonical box) and clip to the box.
    lo, hi = bb[:, 0], bb[:, 1]
    if not (np.allclose(lo, -2.5) and np.allclose(hi, 2.5)):
        points = (points - lo) / (hi - lo) * 5.0 - 2.5
    points = np.clip(points, -2.5, 2.5)

    nc = _build_nc(NT)
    blocks = _host_blocks(velocity)
    pts = _host_points(points)
    in_maps = [{"blocks": blocks[c], "pts": pts[c]} for c in range(NB_CORES)]
    res = run_bass_kernel_spmd(nc, in_maps, core_ids=list(range(NB_CORES)))

    B = velocity.shape[0]
    out = np.empty((B, N, 3), np.float32)
    for core in range(NB_CORES):
        o = res.results[core]["out"]
        for bi in range(B_PER_CORE):
            ob = o[:, bi * L_BATCH : (bi + 1) * L_BATCH, :].reshape(-1, 3)
            out[core * B_PER_CORE + bi] = ob[:N]
    return out
